# revision 1
# baseline (speedup 1.0000x reference)
"""Bass kernel builder for 2-layer LSTM encoder (B=128, T=CHUNK*n_chunks, D=128, H=512).

On-chip layout is "transposed": tiles are [128 partitions, free].
  h.T  -> rolling SBUF buffers [128, (s, j, b)] fp16 (SUB steps per tile)
  c.T  -> SBUF [128, (j, b)] fp32
  gates -> PSUM [128, 2048] fp32, column block mi*128 for m-chunk mi.

m-chunk order (host permutes weight rows to match), j-major:
  mi = [i_j0, f_j0, o_j0, g_j0, i_j1, f_j1, o_j1, g_j1, ...]
"half" hh covers j in {2hh, 2hh+1}: gate columns [hh*1024, (hh+1)*1024).

Roles: cid 0 = layer-1 recurrence, cid 1 = layer-2 recurrence.
If-blocks contain ONLY compute (PE/ACT/DVE) — DMAs inside divergent
branches break DMA-queue semaphore accounting (verified deadlock).
All DMAs and collectives run unconditionally on every core; non-role
cores' h buffers stay zero, so their ReduceScatter contribution is zero.
"""
import sys
sys.path.insert(0, "/opt/trn_rl_repo")
import numpy as np
from concourse import bacc
import concourse.bass as bass
import concourse.mybir as mybir
import concourse.tile as tile

F16 = mybir.dt.float16
F32 = mybir.dt.float32
U32 = mybir.dt.uint32

N_CORES = 8
CORE_IDS = list(range(N_CORES))
P = 128
B = 128
D = 128
H = 512
G4 = 2048
NJ = H // P          # 4
NM = G4 // P         # 16
CHUNK = 32
SHARD = CHUNK // N_CORES  # 4
SUB = 4              # steps per If-block
NSUB = CHUNK // SUB

SIG = mybir.ActivationFunctionType.Sigmoid
TANH = mybir.ActivationFunctionType.Tanh


def build(n_chunks):
    nc = bacc.Bacc()

    # ---------------- inputs ----------------
    xm_in = nc.declare_dram_parameter("x_my", [D, n_chunks * SHARD * B], F16, isOutput=False)
    whh1_in = nc.declare_dram_parameter("whh1", [P, NJ * NM * P], F16, isOutput=False)
    whh2_in = nc.declare_dram_parameter("whh2", [P, NJ * NM * P], F16, isOutput=False)
    wix_in = nc.declare_dram_parameter("wix", [P, NM * P], F16, isOutput=False)
    wox_in = nc.declare_dram_parameter("wox", [P, NJ * NM * P], F16, isOutput=False)
    b1_in = nc.declare_dram_parameter("b1", [P, NM], F32, isOutput=False)
    b2_in = nc.declare_dram_parameter("b2", [P, NM], F32, isOutput=False)
    ident_in = nc.declare_dram_parameter("ident", [P, P], F16, isOutput=False)
    cid_in = nc.declare_dram_parameter("cid", [1, 1], U32, isOutput=False)

    # ---------------- outputs ----------------
    h_out = nc.declare_dram_parameter("h_out", [P, NJ * B], F32, isOutput=True)
    c_out = nc.declare_dram_parameter("c_out", [P, NJ * B], F32, isOutput=True)

    # ---------------- internal DRAM ----------------
    xg1_stage = [nc.dram_tensor(f"xg1_stage{k}", [SHARD, NM, P, B], F16) for k in range(n_chunks)]
    xg1_full = [nc.dram_tensor(f"xg1_full{k}", [CHUNK, NM, P, B], F16, addr_space="Shared") for k in range(n_chunks)]
    xg2_stage = [nc.dram_tensor(f"xg2_stage{k}", [SHARD, NM, P, B], F16) for k in range(n_chunks)]
    xg2_full = [nc.dram_tensor(f"xg2_full{k}", [CHUNK, NM, P, B], F16, addr_space="Shared") for k in range(n_chunks)]
    out1_local = [nc.dram_tensor(f"out1_local{k}", [CHUNK, P, NJ * B], F16) for k in range(n_chunks)]

    with tile.TileContext(nc) as tc:
        with (
            tc.tile_pool(name="wpool", bufs=1) as wpool,
            tc.tile_pool(name="xgb", bufs=2) as xgb,
            tc.tile_pool(name="hrol", bufs=2) as hrol,
            tc.tile_pool(name="state", bufs=1) as state,
            tc.tile_pool(name="actp", bufs=2) as actp,
            tc.tile_pool(name="small", bufs=2) as small,
            tc.tile_pool(name="prod", bufs=2) as prod,
            tc.tile_pool(name="ps", bufs=2, space="PSUM") as psp,
        ):
            # ---- constants ----
            whh1 = wpool.tile([P, NJ * NM * P], F16)
            whh2 = wpool.tile([P, NJ * NM * P], F16)
            wix = wpool.tile([P, NM * P], F16)
            wox = wpool.tile([P, NJ * NM * P], F16)
            b1 = wpool.tile([P, NM], F32)
            b2 = wpool.tile([P, NM], F32)
            ident = wpool.tile([P, P], F16)
            nc.sync.dma_start(out=whh1, in_=whh1_in[:, :])
            nc.sync.dma_start(out=whh2, in_=whh2_in[:, :])
            nc.sync.dma_start(out=wix, in_=wix_in[:, :])
            nc.sync.dma_start(out=wox, in_=wox_in[:, :])
            nc.sync.dma_start(out=b1, in_=b1_in[:, :])
            nc.sync.dma_start(out=b2, in_=b2_in[:, :])
            nc.sync.dma_start(out=ident, in_=ident_in[:, :])
            cid_t = wpool.tile([1, 1], U32)
            nc.sync.dma_start(out=cid_t, in_=cid_in[:, :])
            reg = nc.vector.alloc_register("cid_reg")
            nc.vector.reg_load(reg, cid_t[0:1, 0:1])
            pid = nc.vector.snap(reg, min_val=0, max_val=N_CORES - 1)
            greg = nc.gpsimd.alloc_register("cid_reg_g")
            nc.gpsimd.reg_load(greg, cid_t[0:1, 0:1])
            gpid = nc.gpsimd.snap(greg, min_val=0, max_val=N_CORES - 1)
            off_reg = nc.gpsimd.snap(gpid * SHARD, min_val=0, max_val=CHUNK - SHARD)

            # ---- state ----
            cT1 = state.tile([P, NJ * B], F32)
            cT2 = state.tile([P, NJ * B], F32)
            zero_h = state.tile([P, NJ * B], F16)
            for t_ in (cT1, cT2, zero_h):
                nc.vector.memset(t_, 0.0)

            # ---- produce all xg1 shards upfront + AllGather each ----
            for k in range(n_chunks):
                xs = prod.tile([P, SHARD * B], F16, tag="xs")
                nc.sync.dma_start(out=xs, in_=xm_in[:, k * SHARD * B:(k + 1) * SHARD * B])
                for m in range(NM):
                    pt = psp.tile([P, SHARD * B], F32, tag="gates")
                    nc.tensor.matmul(pt, wix[:, m * P:(m + 1) * P], xs,
                                     start=True, stop=True)
                    sb = prod.tile([P, SHARD * B], F16, tag="xgout")
                    nc.vector.tensor_scalar_add(sb, pt, b1[:, m:m + 1])
                    nc.sync.dma_start(
                        out=xg1_stage[k][:, m, :, :].rearrange("s p b -> p s b"),
                        in_=sb.rearrange("p (s b) -> p s b", s=SHARD))
                nc.gpsimd.collective_compute(
                    "AllGather", mybir.AluOpType.bypass,
                    replica_groups=[CORE_IDS],
                    ins=[xg1_stage[k][:, :, :, :]], outs=[xg1_full[k][:, :, :, :]])

            # ---- one recurrence step, compute only (inside an If) ----
            def step_compute(xg_s, h_prev, h_new, cT, whh):
                """xg_s: [P, NM, B] slice; h_prev/h_new: [P, NJ*B] slices."""
                Gt = psp.tile([P, G4], F32, tag="gates")
                for g in range(4):
                    nc.tensor.matmul(
                        Gt[:, g * 512:(g + 1) * 512], ident,
                        xg_s[:, 4 * g:4 * (g + 1), :],
                        start=True, stop=False, skip_group_check=True)
                for mi in range(NM):
                    for j in range(NJ):
                        nc.tensor.matmul(
                            Gt[:, mi * P:(mi + 1) * P],
                            whh[:, (j * NM + mi) * P:(j * NM + mi + 1) * P],
                            h_prev[:, j * B:(j + 1) * B],
                            start=False, stop=(j == NJ - 1),
                            skip_group_check=True)
                for hh in range(2):
                    Gh = Gt[:, hh * 1024:(hh + 1) * 1024].rearrange(
                        "p (r c) -> p r c", c=512)
                    Sg = actp.tile([P, 2, 3, B], F32, tag=f"S{hh}")
                    nc.scalar.activation(Sg, Gh[:, :, 0:384], SIG)
                    Tg = actp.tile([P, 2, B], F32, tag=f"T{hh}")
                    nc.scalar.activation(Tg, Gh[:, :, 384:512], TANH)
                    cs = cT[:, hh * 2 * B:(hh + 1) * 2 * B]
                    t1 = small.tile([P, 2 * B], F32, tag="t1")
                    nc.vector.tensor_mul(t1, Sg[:, :, 1, :], cs)      # f*c
                    t2 = small.tile([P, 2 * B], F32, tag="t2")
                    nc.vector.tensor_mul(t2, Sg[:, :, 0, :], Tg)      # i*tanh(g)
                    nc.vector.tensor_add(cs, t1, t2)
                    tcv = small.tile([P, 2 * B], F32, tag="tc")
                    nc.scalar.activation(tcv, cs, TANH)
                    nc.vector.tensor_mul(
                        h_new[:, hh * 2 * B:(hh + 1) * 2 * B], Sg[:, :, 2, :], tcv)

            # rolling h buffers; element [p, (s, j, b)]
            hb_prev = {1: None, 2: None}

            def sub_block(layer, k, u, cT, whh, xg_full):
                """SUB steps; returns the new rolling h tile."""
                xgt = xgb.tile([P, SUB, NM, B], F16, tag=f"xg{layer}")
                dma_eng = nc.sync if layer == 1 else nc.scalar
                dma_eng.dma_start(
                    out=xgt,
                    in_=xg_full[k][u * SUB:(u + 1) * SUB, :, :, :].rearrange(
                        "s m p b -> p s m b"))
                hb = hrol.tile([P, SUB, NJ * B], F16, tag=f"h{layer}")
                prev = hb_prev[layer]
                for s in range(SUB):
                    h_prev = (zero_h[:, :] if prev is None and s == 0
                              else (prev[:, SUB - 1, :] if s == 0
                                    else hb[:, s - 1, :]))
                    step_compute(xgt[:, s, :, :], h_prev, hb[:, s, :],
                                 cT, whh)
                hb_prev[layer] = hb
                return hb

            def produce_xg2(k):
                oh = prod.tile([P, SHARD, NJ * B], F16, tag="oh")
                nc.gpsimd.dma_start(
                    out=oh,
                    in_=out1_local[k][bass.ds(off_reg, SHARD)].rearrange(
                        "s p f -> p s f"))
                for m in range(NM):
                    pt = psp.tile([P, SHARD * B], F32, tag="gates")
                    for j in range(NJ):
                        nc.tensor.matmul(
                            pt, wox[:, (j * NM + m) * P:(j * NM + m + 1) * P],
                            oh[:, :, j * B:(j + 1) * B],
                            start=(j == 0), stop=(j == NJ - 1))
                    sb = prod.tile([P, SHARD * B], F16, tag="xgout")
                    nc.vector.tensor_scalar_add(sb, pt, b2[:, m:m + 1])
                    nc.gpsimd.dma_start(
                        out=xg2_stage[k][:, m, :, :].rearrange("s p b -> p s b"),
                        in_=sb.rearrange("p (s b) -> p s b", s=SHARD))

            def l2_chunk(k):
                nc.gpsimd.collective_compute(
                    "AllGather", mybir.AluOpType.bypass,
                    replica_groups=[CORE_IDS],
                    ins=[xg2_stage[k][:, :, :, :]], outs=[xg2_full[k][:, :, :, :]])
                for u in range(NSUB):
                    sub_block(2, k, u, cT2, whh2, xg2_full)

            # ---- pipeline over chunks (L2 lags by one chunk) ----
            for k in range(n_chunks):
                for u in range(NSUB):
                    hb = sub_block(1, k, u, cT1, whh1, xg1_full)
                    nc.gpsimd.dma_start(
                        out=out1_local[k][u * SUB:(u + 1) * SUB].rearrange(
                            "s p f -> p s f"),
                        in_=hb)
                produce_xg2(k)
                if k > 0:
                    l2_chunk(k - 1)
            l2_chunk(n_chunks - 1)

            # ---- final outputs (identical on all cores) ----
            h32 = state.tile([P, NJ * B], F32)
            nc.vector.tensor_copy(h32, hb_prev[2][:, SUB - 1, :])
            nc.sync.dma_start(out=h_out[:, :], in_=h32)
            nc.sync.dma_start(out=c_out[:, :], in_=cT2)
    return nc


# ---------------- host-side packing ----------------

def _perm_rows():
    gate_base = {"i": 0, "f": H, "g": 2 * H, "o": 3 * H}
    order = []
    for j in range(NJ):
        for gname in ("i", "f", "o", "g"):
            start = gate_base[gname] + j * P
            order.extend(range(start, start + P))
    return np.array(order)


def pack_inputs(x, W_ih1, W_hh1, b_ih1, b_hh1, W_ih2, W_hh2, b_ih2, b_hh2,
                n_chunks):
    perm = _perm_rows()
    W_ih1 = np.asarray(W_ih1)[perm]
    W_hh1 = np.asarray(W_hh1)[perm]
    W_ih2 = np.asarray(W_ih2)[perm]
    W_hh2 = np.asarray(W_hh2)[perm]
    bias1 = (np.asarray(b_ih1) + np.asarray(b_hh1))[perm].astype(np.float32)
    bias2 = (np.asarray(b_ih2) + np.asarray(b_hh2))[perm].astype(np.float32)

    def pack_whh(W):
        Wr = W.reshape(NM, P, NJ, P)       # [mi, q, j, p]
        out = Wr.transpose(3, 2, 0, 1)     # [p, j, mi, q]
        return np.ascontiguousarray(out.reshape(P, NJ * NM * P)).astype(np.float16)

    whh1 = pack_whh(W_hh1)
    whh2 = pack_whh(W_hh2)
    wox = pack_whh(W_ih2)
    wix = np.ascontiguousarray(
        W_ih1.reshape(NM, P, D).transpose(2, 0, 1).reshape(D, NM * P)).astype(np.float16)

    b1p = np.ascontiguousarray(bias1.reshape(NM, P).T)
    b2p = np.ascontiguousarray(bias2.reshape(NM, P).T)
    ident = np.eye(P, dtype=np.float16)

    x16 = np.asarray(x).astype(np.float16)
    in_maps = []
    for r in range(N_CORES):
        steps = np.concatenate([
            np.arange(k * CHUNK + r * SHARD, k * CHUNK + r * SHARD + SHARD)
            for k in range(n_chunks)])
        xs = x16[:, steps, :]
        xm = np.ascontiguousarray(xs.transpose(2, 1, 0).reshape(D, n_chunks * SHARD * B))
        in_maps.append({
            "x_my": xm,
            "whh1": whh1, "whh2": whh2, "wix": wix, "wox": wox,
            "b1": b1p, "b2": b2p, "ident": ident,
            "cid": np.array([[r]], np.uint32),
        })
    return in_maps


def unpack_outputs(results):
    hT = results[0]["h_out"]
    cT = results[0]["c_out"]
    def un(a):
        return np.ascontiguousarray(
            a.reshape(P, NJ, B).transpose(2, 1, 0).reshape(B, H))
    return un(hT)[None], un(cT)[None]


# ---------------- harness entry point ----------------

N_CHUNKS_FULL = 16  # T = 512

_CACHE = {}


def _get_nc():
    if "nc" not in _CACHE:
        nc = build(N_CHUNKS_FULL)
        nc.finalize()
        _CACHE["nc"] = nc
    return _CACHE["nc"]


def kernel(x, W_ih1, W_hh1, b_ih1, b_hh1, W_ih2, W_hh2, b_ih2, b_hh2):
    """Full (unsharded) inputs -> full output, matching reference():
    returns (h_T [1, B, H], c_T [1, B, H]) of layer 2, fp32."""
    import time as _time
    from concourse.bass_utils import run_bass_kernel_spmd
    nc = _get_nc()
    in_maps = pack_inputs(x, W_ih1, W_hh1, b_ih1, b_hh1,
                          W_ih2, W_hh2, b_ih2, b_hh2, N_CHUNKS_FULL)
    last_err = None
    for attempt in range(3):
        try:
            res = run_bass_kernel_spmd(nc, in_maps, CORE_IDS)
            h, c = unpack_outputs(res.results)
            return h.astype(np.float32), c.astype(np.float32)
        except Exception as e:  # transient device wedge: back off and retry
            last_err = e
            _time.sleep(5 * (attempt + 1))
    raise last_err

